# revision 1
# baseline (speedup 1.0000x reference)
"""Grouped-experts MoE (SwiGLU) Bass kernel for Trainium2, 8 NeuronCores.

Expert-parallel: core c owns experts [8c, 8c+8). Tokens are pre-grouped by
expert in the input, so routing is host-side slicing. All device matmuls run
in transposed-token space so every operand streams in its natural layout:

  gateT[i, t] = sum_k G[k, i] * xT[k, t]      (lhsT = G tile, rhs = xT tile)
  hT = silu(gateT) * upT                       (elementwise, [inter, tok])
  outT[m, t] = sum_ki D[ki, m] * hT[ki, t]     (lhsT = D tile, rhs = hT tile)

Host transposes x in / out once per core (not on the device clock).
Compute in bf16 with fp32 PSUM accumulation.
"""

import numpy as np
import ml_dtypes

NUM_EXPERTS = 64
HID = 2048
INTER = 768
N_CORES = 8
EPC = NUM_EXPERTS // N_CORES  # experts per core
KT = HID // 128    # 16 k-tiles over hidden
IT = INTER // 128  # 6 tiles over intermediate
CHUNK = 512        # moving-operand free dim per matmul
SEG_MAX = 1280     # max tokens handled per weight-load segment

BF16_NP = ml_dtypes.bfloat16

_cache = {}


def _chunks(p):
    """Balanced split into ceil(p/CHUNK) near-equal chunks (keeps every
    matmul moving dim wide enough to hide LDWEIGHTS)."""
    if p <= 0:
        return []
    nch = -(-p // CHUNK)
    base, rem = divmod(p, nch)
    out = []
    n0 = 0
    for i in range(nch):
        sz = base + (1 if i < rem else 0)
        out.append((n0, sz))
        n0 += sz
    return out


def _segments(padded):
    """[(slot_idx, col_offset, seg_len)] with seg_len <= SEG_MAX."""
    segs = []
    off = 0
    for j, p in enumerate(padded):
        done = 0
        while done < p:
            take = min(SEG_MAX, p - done)
            segs.append((j, off + done, take))
            done += take
        off += p
    return segs


def _build(padded):
    import concourse.bacc as bacc
    import concourse.mybir as mybir
    import concourse.tile as tile

    BF16 = mybir.dt.bfloat16
    F32 = mybir.dt.float32
    SILU = mybir.ActivationFunctionType.Silu

    ptot = int(sum(padded))
    segs = _segments(padded)
    max_seg = max(s[2] for s in segs)

    nc = bacc.Bacc("TRN2", target_bir_lowering=False, debug=False,
                   num_devices=N_CORES)

    xt = nc.dram_tensor("xt", [HID, ptot], BF16, kind="ExternalInput")
    gw = nc.dram_tensor("gw", [EPC, HID, INTER], BF16, kind="ExternalInput")
    uw = nc.dram_tensor("uw", [EPC, HID, INTER], BF16, kind="ExternalInput")
    dw = nc.dram_tensor("dw", [EPC, INTER, HID], BF16, kind="ExternalInput")
    yt = nc.dram_tensor("yt", [HID, ptot], F32, kind="ExternalOutput")

    # SBUF budget (bytes/partition, 192K usable):
    #   x    17 x max_seg*2   (43.5K at 1280)
    #   w    44 x 1536        (66K)   G/U k-rows [128, INTER]
    #   d     8 x 4096        (32K)
    #   h    20 x 1024        (20K)
    #   silu  4 x 2048        (8K)
    #   out   6 x 2048        (12K)
    bx = max(17, min(24, (44 * 1024) // (max_seg * 2)))

    with tile.TileContext(nc) as tc:
        with (
            tc.tile_pool(name="xp", bufs=bx) as xp,
            tc.tile_pool(name="wp", bufs=44) as wp,
            tc.tile_pool(name="dp", bufs=8) as dp,
            tc.tile_pool(name="hp", bufs=20) as hp,
            tc.tile_pool(name="sp", bufs=4) as sp,
            tc.tile_pool(name="op", bufs=6) as op,
            tc.tile_pool(name="psg", bufs=2, space="PSUM") as psg,
            tc.tile_pool(name="psu", bufs=2, space="PSUM") as psu,
            tc.tile_pool(name="psd", bufs=4, space="PSUM") as psd,
        ):
            for (slot, col0, seg) in segs:
                ch = _chunks(seg)

                # Loads in consumption order on the in-order sync ring:
                # (G_k, x_k) pairs feed the first matmuls ASAP, U next, D last.
                gt, ut, xk = [], [], []
                for k in range(KT):
                    g = wp.tile([128, INTER], BF16, tag="w")
                    nc.sync.dma_start(g[:], gw[slot, k * 128:(k + 1) * 128, :])
                    gt.append(g)
                    t = xp.tile([128, seg], BF16, tag="x")
                    nc.sync.dma_start(t[:], xt[k * 128:(k + 1) * 128,
                                                col0:col0 + seg])
                    xk.append(t)
                for k in range(KT):
                    u = wp.tile([128, INTER], BF16, tag="w")
                    nc.sync.dma_start(u[:], uw[slot, k * 128:(k + 1) * 128, :])
                    ut.append(u)
                dk = []
                for ki in range(IT):
                    d = dp.tile([128, HID], BF16, tag="d")
                    nc.sync.dma_start(d[:], dw[slot, ki * 128:(ki + 1) * 128, :])
                    dk.append(d)

                # ---- gate/up phase ----
                h = {}
                for i in range(IT):
                    for ci, (n0, nsz) in enumerate(ch):
                        pg = psg.tile([128, nsz], F32, tag="pg")
                        for k in range(KT):
                            nc.tensor.matmul(pg[:],
                                             gt[k][:, i * 128:(i + 1) * 128],
                                             xk[k][:, n0:n0 + nsz],
                                             start=(k == 0), stop=(k == KT - 1))
                        pu = psu.tile([128, nsz], F32, tag="pu")
                        for k in range(KT):
                            nc.tensor.matmul(pu[:],
                                             ut[k][:, i * 128:(i + 1) * 128],
                                             xk[k][:, n0:n0 + nsz],
                                             start=(k == 0), stop=(k == KT - 1))
                        st = sp.tile([128, nsz], F32, tag="s")
                        nc.scalar.activation(st[:], pg[:], SILU)
                        ht = hp.tile([128, nsz], BF16, tag="h")
                        nc.vector.tensor_mul(ht[:], st[:], pu[:])
                        h[(i, ci)] = ht

                # ---- down phase ----
                for ci, (n0, nsz) in enumerate(ch):
                    for m in range(KT):
                        pd = psd.tile([128, nsz], F32, tag="pd")
                        for ki in range(IT):
                            nc.tensor.matmul(pd[:],
                                             dk[ki][:, m * 128:(m + 1) * 128],
                                             h[(ki, ci)][:],
                                             start=(ki == 0), stop=(ki == IT - 1))
                        ot = op.tile([128, nsz], F32, tag="o")
                        if m % 2 == 0:
                            nc.scalar.copy(ot[:], pd[:])
                        else:
                            nc.vector.tensor_copy(ot[:], pd[:])
                        nc.gpsimd.dma_start(
                            yt[m * 128:(m + 1) * 128,
                               col0 + n0:col0 + n0 + nsz], ot[:])

    nc.compile()
    return nc, ptot


def _get_program(padded):
    key = tuple(padded)
    if key not in _cache:
        _cache[key] = _build(padded)
    return _cache[key]


def _invoke(x, gate_proj, up_proj, down_proj, num_tokens_per_expert,
            trace=False, trace_kwargs=None):
    from concourse.bass_utils import run_bass_kernel_spmd

    x = np.asarray(x)
    counts = np.asarray(num_tokens_per_expert).astype(np.int64)
    assert counts.shape == (NUM_EXPERTS,)
    starts = np.zeros(NUM_EXPERTS + 1, dtype=np.int64)
    np.cumsum(counts, out=starts[1:])

    # per-slot padded counts (max over cores) -> one SPMD program
    cmat = counts.reshape(N_CORES, EPC)
    padded = [int(cmat[:, j].max()) for j in range(EPC)]
    offs = np.zeros(EPC + 1, dtype=np.int64)
    np.cumsum(np.asarray(padded), out=offs[1:])
    ptot_expected = int(offs[-1])

    nc, ptot = _get_program(padded)
    assert ptot == ptot_expected

    gb = np.asarray(gate_proj).astype(BF16_NP)
    ub = np.asarray(up_proj).astype(BF16_NP)
    db = np.asarray(down_proj).astype(BF16_NP)

    in_maps = []
    for c in range(N_CORES):
        xtc = np.zeros((HID, ptot), dtype=BF16_NP)
        for j in range(EPC):
            e = c * EPC + j
            cnt = int(counts[e])
            if cnt:
                xtc[:, int(offs[j]):int(offs[j]) + cnt] = \
                    x[int(starts[e]):int(starts[e]) + cnt].astype(BF16_NP).T
        in_maps.append({
            "xt": xtc,
            "gw": gb[c * EPC:(c + 1) * EPC],
            "uw": ub[c * EPC:(c + 1) * EPC],
            "dw": db[c * EPC:(c + 1) * EPC],
        })

    res = run_bass_kernel_spmd(nc, in_maps, list(range(N_CORES)),
                               trace=trace, **(trace_kwargs or {}))

    out = np.empty((int(starts[-1]), HID), dtype=np.float32)
    for c in range(N_CORES):
        ytc = res.results[c]["yt"]
        for j in range(EPC):
            e = c * EPC + j
            cnt = int(counts[e])
            if cnt:
                out[int(starts[e]):int(starts[e]) + cnt] = \
                    ytc[:, int(offs[j]):int(offs[j]) + cnt].T
    return out, res


def kernel(x, gate_proj, up_proj, down_proj, num_tokens_per_expert):
    out, _ = _invoke(x, gate_proj, up_proj, down_proj, num_tokens_per_expert)
    return out



# revision 6
# speedup vs baseline: 1.0157x; 1.0157x over previous
"""Grouped-experts MoE (SwiGLU) Bass kernel for Trainium2, 8 NeuronCores.

Expert-parallel: core c owns experts [8c, 8c+8). Tokens are pre-grouped by
expert in the input, so routing is host-side slicing. All device matmuls run
in transposed-token space so every operand streams in its natural layout:

  gateT[i, t] = sum_k G[k, i] * xT[k, t]      (lhsT = G tile, rhs = xT tile)
  hT = silu(gateT) * upT                       (elementwise, [inter, tok])
  outT[m, t] = sum_ki D[ki, m] * hT[ki, t]     (lhsT = D tile, rhs = hT tile)

v2 scheduling (vs v1 baseline at ~1094us):
  - tokens processed in 512-wide chunks; x streamed per-chunk (short tile
    lifetimes -> real prefetch with modest SBUF)
  - three DMA queues in parallel: sync=weights, scalar=x, gpsimd=outputs
    (v1 put all inputs on one in-order sync queue -> PE stalls at segment
    starts waiting behind weight bursts)
  - down-projection groups of chunk t-1 interleave between gate/up pairs of
    chunk t -> no phase-boundary bubbles, output DMA spread evenly
  - bf16 output (halves store traffic; rel-err impact ~1e-4)
  - cold start: expert-0 gate weights split across sync+scalar queues,
    first x chunk on gpsimd -> first PSUM group completes ~6us in
Host transposes x in / out once per core (not on the device clock).
Compute in bf16 with fp32 PSUM accumulation.
"""

import numpy as np
import ml_dtypes

NUM_EXPERTS = 64
HID = 2048
INTER = 768
N_CORES = 8
EPC = NUM_EXPERTS // N_CORES  # experts per core
KT = HID // 128    # 16 k-tiles over hidden
IT = INTER // 128  # 6 tiles over intermediate
CHUNK = 512        # moving-operand free dim per matmul (HW max)

BF16_NP = ml_dtypes.bfloat16

_cache = {}


def _chunks_of(p):
    """Split p tokens into 512-token chunks, remainder last."""
    out = []
    n0 = 0
    while p - n0 > 0:
        take = min(CHUNK, p - n0)
        out.append((n0, take))
        n0 += take
    return out


def _chunk_list(padded):
    """[(slot, col0, n, first_of_expert)] over all experts of this core."""
    ch = []
    off = 0
    for j, p in enumerate(padded):
        for idx, (n0, n) in enumerate(_chunks_of(p)):
            ch.append((j, off + n0, n, idx == 0))
        off += p
    return ch


def _build(padded):
    import concourse.bacc as bacc
    import concourse.mybir as mybir
    import concourse.tile as tile

    BF16 = mybir.dt.bfloat16
    F32 = mybir.dt.float32
    SILU = mybir.ActivationFunctionType.Silu

    ptot = int(sum(padded))
    CH = _chunk_list(padded)
    NCH = len(CH)

    nc = bacc.Bacc("TRN2", target_bir_lowering=False, debug=False,
                   num_devices=N_CORES)

    xt = nc.dram_tensor("xt", [HID, ptot], BF16, kind="ExternalInput")
    gw = nc.dram_tensor("gw", [EPC, HID, INTER], BF16, kind="ExternalInput")
    uw = nc.dram_tensor("uw", [EPC, HID, INTER], BF16, kind="ExternalInput")
    dw = nc.dram_tensor("dw", [EPC, INTER, HID], BF16, kind="ExternalInput")
    yt = nc.dram_tensor("yt", [HID, ptot], BF16, kind="ExternalOutput")

    # cumulative down-group counts emitted after gate/up pair j (j=0..5)
    DOWN_SCHED = [0, 3, 6, 10, 13, 16]

    with tile.TileContext(nc) as tc:
        with (
            tc.tile_pool(name="xp", bufs=36) as xp,    # 36K/part
            tc.tile_pool(name="gp", bufs=32) as gp,    # 48K
            tc.tile_pool(name="upl", bufs=28) as upl,  # 42K
            tc.tile_pool(name="dp", bufs=8) as dp,     # 32K
            tc.tile_pool(name="hp", bufs=14) as hp,    # 14K
            tc.tile_pool(name="sp", bufs=4) as sp,     # 8K
            tc.tile_pool(name="op", bufs=8) as op,     # 8K
            tc.tile_pool(name="psg", bufs=2, space="PSUM") as psg,
            tc.tile_pool(name="psu", bufs=2, space="PSUM") as psu,
            tc.tile_pool(name="psd", bufs=3, space="PSUM") as psd,
        ):
            gt = {}   # expert -> [16 G k-tiles]
            ut = {}
            dk = {}
            xtl = {}  # chunk idx -> [16 x k-tiles]

            def load_weights(e, queue):
                g = []
                for k in range(KT):
                    w = gp.tile([128, INTER], BF16, tag="g", name=f"g{e}_{k}")
                    queue.dma_start(w[:], gw[e, k * 128:(k + 1) * 128, :])
                    g.append(w)
                gt[e] = g
                u = []
                for k in range(KT):
                    w = upl.tile([128, INTER], BF16, tag="u", name=f"u{e}_{k}")
                    queue.dma_start(w[:], uw[e, k * 128:(k + 1) * 128, :])
                    u.append(w)
                ut[e] = u
                d = []
                for ki in range(IT):
                    w = dp.tile([128, HID], BF16, tag="d", name=f"d{e}_{ki}")
                    queue.dma_start(w[:], dw[e, ki * 128:(ki + 1) * 128, :])
                    d.append(w)
                dk[e] = d

            def load_x(t, queue):
                slot, col0, n, _ = CH[t]
                xs = []
                for k in range(KT):
                    x = xp.tile([128, n], BF16, tag="x", name=f"x{t}_{k}",
                                padded_shape=[128, CHUNK])
                    queue.dma_start(x[:], xt[k * 128:(k + 1) * 128,
                                              col0:col0 + n])
                    xs.append(x)
                xtl[t] = xs

            # ---- cold-start prologue ----
            # expert-0 G split across sync+scalar; first x chunk on gpsimd so
            # all three rings fill the first PSUM group in parallel.
            g0 = []
            for k in range(KT):
                w = gp.tile([128, INTER], BF16, tag="g", name=f"g0_{k}")
                q = nc.sync if k % 2 == 0 else nc.scalar
                q.dma_start(w[:], gw[0, k * 128:(k + 1) * 128, :])
                g0.append(w)
            gt[0] = g0
            load_x(0, nc.gpsimd)
            u0 = []
            for k in range(KT):
                w = upl.tile([128, INTER], BF16, tag="u", name=f"u0_{k}")
                q = nc.sync if k % 2 == 0 else nc.scalar
                q.dma_start(w[:], uw[0, k * 128:(k + 1) * 128, :])
                u0.append(w)
            ut[0] = u0
            d0 = []
            for ki in range(IT):
                w = dp.tile([128, HID], BF16, tag="d", name=f"d0_{ki}")
                nc.sync.dma_start(w[:], dw[0, ki * 128:(ki + 1) * 128, :])
                d0.append(w)
            dk[0] = d0
            load_x(1, nc.scalar)

            h = {}          # (chunk, i) -> h tile
            pend = None     # chunk whose down-groups still need emitting
            emitted = 0     # down-groups of `pend` already emitted

            def down_group(t, m):
                slot, col0, n, _ = CH[t]
                e = slot
                pd = psd.tile([128, n], F32, tag="pd",
                              padded_shape=[128, CHUNK])
                for ki in range(IT):
                    nc.tensor.matmul(pd[:],
                                     dk[e][ki][:, m * 128:(m + 1) * 128],
                                     h[(t, ki)][:],
                                     start=(ki == 0), stop=(ki == IT - 1))
                ot = op.tile([128, n], BF16, tag="o",
                             padded_shape=[128, CHUNK])
                nc.vector.tensor_copy(ot[:], pd[:])
                nc.gpsimd.dma_start(
                    yt[m * 128:(m + 1) * 128, col0:col0 + n], ot[:])

            for t in range(NCH):
                slot, col0, n, first = CH[t]
                e = slot

                # weight prefetch for the next expert (sync queue only —
                # nothing else rides it, so head-of-line waits are harmless)
                if first and e + 1 < EPC:
                    load_weights(e + 1, nc.sync)

                # gate/up pairs with down-groups of the previous chunk
                # interleaved between pairs
                for i in range(IT):
                    pg = psg.tile([128, n], F32, tag="pg",
                                  padded_shape=[128, CHUNK])
                    for k in range(KT):
                        nc.tensor.matmul(pg[:],
                                         gt[e][k][:, i * 128:(i + 1) * 128],
                                         xtl[t][k][:],
                                         start=(k == 0), stop=(k == KT - 1))
                    pu = psu.tile([128, n], F32, tag="pu",
                                  padded_shape=[128, CHUNK])
                    for k in range(KT):
                        nc.tensor.matmul(pu[:],
                                         ut[e][k][:, i * 128:(i + 1) * 128],
                                         xtl[t][k][:],
                                         start=(k == 0), stop=(k == KT - 1))
                    st = sp.tile([128, n], F32, tag="s",
                                 padded_shape=[128, CHUNK])
                    nc.scalar.activation(st[:], pg[:], SILU)
                    ht = hp.tile([128, n], BF16, tag="h",
                                 padded_shape=[128, CHUNK])
                    nc.vector.tensor_mul(ht[:], st[:], pu[:])
                    h[(t, i)] = ht

                    if pend is not None:
                        while emitted < DOWN_SCHED[i]:
                            down_group(pend, emitted)
                            emitted += 1

                # x prefetch AFTER this chunk's silus: the x DMAs may wait on
                # x(t) tile frees (u-group(t,5)); emitting them before the
                # silus would deadlock scalar-queue head-of-line against the
                # PSUM-bank WAR dependency (silu -> g-group(t,i+2)).
                if t + 2 < NCH:
                    load_x(t + 2, nc.scalar)

                if pend is not None:
                    while emitted < KT:
                        down_group(pend, emitted)
                        emitted += 1
                    for ki in range(IT):
                        del h[(pend, ki)]
                pend = t
                emitted = 0

            # tail: down-groups of the final chunk
            for m in range(KT):
                down_group(pend, m)

    nc.compile()
    return nc, ptot


def _get_program(padded):
    key = tuple(padded)
    if key not in _cache:
        _cache[key] = _build(padded)
    return _cache[key]


def _invoke(x, gate_proj, up_proj, down_proj, num_tokens_per_expert,
            trace=False, trace_kwargs=None):
    from concourse.bass_utils import run_bass_kernel_spmd

    x = np.asarray(x)
    counts = np.asarray(num_tokens_per_expert).astype(np.int64)
    assert counts.shape == (NUM_EXPERTS,)
    starts = np.zeros(NUM_EXPERTS + 1, dtype=np.int64)
    np.cumsum(counts, out=starts[1:])

    # per-slot padded counts (max over cores) -> one SPMD program
    cmat = counts.reshape(N_CORES, EPC)
    padded = [int(cmat[:, j].max()) for j in range(EPC)]
    offs = np.zeros(EPC + 1, dtype=np.int64)
    np.cumsum(np.asarray(padded), out=offs[1:])
    ptot_expected = int(offs[-1])

    nc, ptot = _get_program(padded)
    assert ptot == ptot_expected

    gb = np.asarray(gate_proj).astype(BF16_NP)
    ub = np.asarray(up_proj).astype(BF16_NP)
    db = np.asarray(down_proj).astype(BF16_NP)

    in_maps = []
    for c in range(N_CORES):
        xtc = np.zeros((HID, ptot), dtype=BF16_NP)
        for j in range(EPC):
            e = c * EPC + j
            cnt = int(counts[e])
            if cnt:
                xtc[:, int(offs[j]):int(offs[j]) + cnt] = \
                    x[int(starts[e]):int(starts[e]) + cnt].astype(BF16_NP).T
        in_maps.append({
            "xt": xtc,
            "gw": gb[c * EPC:(c + 1) * EPC],
            "uw": ub[c * EPC:(c + 1) * EPC],
            "dw": db[c * EPC:(c + 1) * EPC],
        })

    res = run_bass_kernel_spmd(nc, in_maps, list(range(N_CORES)),
                               trace=trace, **(trace_kwargs or {}))

    out = np.empty((int(starts[-1]), HID), dtype=np.float32)
    for c in range(N_CORES):
        ytc = res.results[c]["yt"]
        for j in range(EPC):
            e = c * EPC + j
            cnt = int(counts[e])
            if cnt:
                out[int(starts[e]):int(starts[e]) + cnt] = \
                    ytc[:, int(offs[j]):int(offs[j]) + cnt].T \
                    .astype(np.float32)
    return out, res


def kernel(x, gate_proj, up_proj, down_proj, num_tokens_per_expert):
    out, _ = _invoke(x, gate_proj, up_proj, down_proj, num_tokens_per_expert)
    return out


# revision 9
# speedup vs baseline: 1.0292x; 1.0133x over previous
"""Grouped-experts MoE (SwiGLU) Bass kernel for Trainium2, 8 NeuronCores.

Expert-parallel: core c owns experts [8c, 8c+8). Tokens are pre-grouped by
expert in the input, so routing is host-side slicing. All device matmuls run
in transposed-token space so every operand streams in its natural layout:

  gateT[i, t] = sum_k G[k, i] * xT[k, t]      (lhsT = G tile, rhs = xT tile)
  hT = silu(gateT) * upT                       (elementwise, [inter, tok])
  outT[m, t] = sum_ki D[ki, m] * hT[ki, t]     (lhsT = D tile, rhs = hT tile)

v3 scheduling (v1 ~1094us, v2 ~1077us):
  - batched mega-DMAs via 3D access patterns: ONE dma per weight matrix per
    expert and ONE per 512-token x chunk (DMA rings process each dma_start
    serially at ~0.6us issue cost; v2's 16-instruction bursts paced the PE)
  - all weights fully double-buffered (G/U/D tiles never wait on frees ->
    no expert-boundary stalls)
  - per chunk: gate-phase (6 groups) then up-phase (6 groups): U(0) only
    needed ~20us after first matmul -> shorter cold start; silu overlaps
    g-phase, mul overlaps u-phase
  - down-projection groups of chunk t-1 interleaved 2-at-a-time between
    groups of chunk t; outputs copied to bf16 and stored in 4-m-group
    batched DMAs
  - psg has 3 PSUM banks so a slow silu (stuck behind an x DMA issue on the
    scalar queue) can't stall the PE; 3+2+3 = 8 banks used
  - first expert processes its remainder chunk first (smaller cold-start
    footprint); last expert ends with two 128-token chunks (short tail)
Host transposes x in / out once per core (not on the device clock).
Compute in bf16 with fp32 PSUM accumulation; bf16 output.
"""

import numpy as np
import ml_dtypes

NUM_EXPERTS = 64
HID = 2048
INTER = 768
N_CORES = 8
EPC = NUM_EXPERTS // N_CORES  # experts per core
KT = HID // 128    # 16 k-tiles over hidden
IT = INTER // 128  # 6 tiles over intermediate
CHUNK = 512        # moving-operand free dim per matmul (HW max)

BF16_NP = ml_dtypes.bfloat16

_cache = {}


def _chunks_of(p, first_expert, last_expert):
    """Chunk sizes for one expert's p tokens."""
    sizes = []
    full, rem = divmod(p, CHUNK)
    if first_expert:
        if rem:
            sizes.append(rem)
        sizes += [CHUNK] * full
    else:
        sizes += [CHUNK] * full
        if rem:
            sizes.append(rem)
    if last_expert and sizes and sizes[-1] > 128:
        # split the tail so the final (non-interleavable) down-phase is short
        tail = sizes.pop()
        sizes += [tail - 128, 128]
    out = []
    n0 = 0
    for s in sizes:
        out.append((n0, s))
        n0 += s
    return out


def _chunk_list(padded):
    """[(slot, col0, n, first_of_expert)] over all experts of this core."""
    ch = []
    off = 0
    ne = len(padded)
    for j, p in enumerate(padded):
        for idx, (n0, n) in enumerate(
                _chunks_of(p, j == 0, j == ne - 1)):
            ch.append((j, off + n0, n, idx == 0))
        off += p
    return ch


def _build(padded):
    import concourse.bacc as bacc
    import concourse.mybir as mybir
    import concourse.tile as tile

    BF16 = mybir.dt.bfloat16
    F32 = mybir.dt.float32
    SILU = mybir.ActivationFunctionType.Silu

    ptot = int(sum(padded))
    CH = _chunk_list(padded)
    NCH = len(CH)

    nc = bacc.Bacc("TRN2", target_bir_lowering=False, debug=False,
                   num_devices=N_CORES)

    xt = nc.dram_tensor("xt", [HID, ptot], BF16, kind="ExternalInput")
    gw = nc.dram_tensor("gw", [EPC, HID, INTER], BF16, kind="ExternalInput")
    uw = nc.dram_tensor("uw", [EPC, HID, INTER], BF16, kind="ExternalInput")
    dw = nc.dram_tensor("dw", [EPC, INTER, HID], BF16, kind="ExternalInput")
    yt = nc.dram_tensor("yt", [HID, ptot], BF16, kind="ExternalOutput")

    with tile.TileContext(nc) as tc:
        with (
            tc.tile_pool(name="xp", bufs=2) as xp,     # 2 x 16K/part
            tc.tile_pool(name="gp", bufs=2) as gp,     # 2 x 24K
            tc.tile_pool(name="upl", bufs=2) as upl,   # 2 x 24K
            tc.tile_pool(name="dp", bufs=2) as dp,     # 2 x 24K
            tc.tile_pool(name="hp", bufs=12) as hp,    # 12K
            tc.tile_pool(name="sp", bufs=6) as sp,     # 12K
            tc.tile_pool(name="op", bufs=2) as op,     # 2 x 4K
            tc.tile_pool(name="psg", bufs=3, space="PSUM") as psg,
            tc.tile_pool(name="psu", bufs=2, space="PSUM") as psu,
            tc.tile_pool(name="psd", bufs=3, space="PSUM") as psd,
        ):
            gt = {}   # expert -> [128, KT, INTER] tile
            ut = {}
            dk = {}   # expert -> [128, IT, HID] tile
            xtl = {}  # chunk idx -> [128, KT, n] tile

            def gsrc(e):
                return gw[e].rearrange("(k p) i -> p k i", p=128)

            def usrc(e):
                return uw[e].rearrange("(k p) i -> p k i", p=128)

            def dsrc(e):
                return dw[e].rearrange("(k p) m -> p k m", p=128)

            def load_weights(e, queue):
                g = gp.tile([128, KT, INTER], BF16, tag="g", name=f"g{e}")
                queue.dma_start(g[:], gsrc(e))
                gt[e] = g
                u = upl.tile([128, KT, INTER], BF16, tag="u", name=f"u{e}")
                queue.dma_start(u[:], usrc(e))
                ut[e] = u
                d = dp.tile([128, IT, HID], BF16, tag="d", name=f"d{e}")
                queue.dma_start(d[:], dsrc(e))
                dk[e] = d

            def load_x(t, queue):
                slot, col0, n, _ = CH[t]
                x = xp.tile([128, KT, n], BF16, tag="x", name=f"x{t}",
                            padded_shape=[128, KT, CHUNK])
                queue.dma_start(
                    x[:], xt[:, col0:col0 + n].rearrange(
                        "(k p) n -> p k n", p=128))
                xtl[t] = x

            # ---- cold-start prologue ----
            # Spread the first expert's working set over all three trigger
            # queues roughly in need-by order.
            g0 = gp.tile([128, KT, INTER], BF16, tag="g", name="g0")
            gs = gsrc(0)
            nc.sync.dma_start(g0[:, 0:6, :], gs[:, 0:6, :])
            nc.scalar.dma_start(g0[:, 6:11, :], gs[:, 6:11, :])
            nc.gpsimd.dma_start(g0[:, 11:KT, :], gs[:, 11:KT, :])
            gt[0] = g0
            load_x(0, nc.gpsimd)
            u0 = upl.tile([128, KT, INTER], BF16, tag="u", name="u0")
            us = usrc(0)
            nc.scalar.dma_start(u0[:, 0:8, :], us[:, 0:8, :])
            nc.sync.dma_start(u0[:, 8:KT, :], us[:, 8:KT, :])
            ut[0] = u0
            load_x(1, nc.sync)
            d0 = dp.tile([128, IT, HID], BF16, tag="d", name="d0")
            nc.gpsimd.dma_start(d0[:], dsrc(0))
            dk[0] = d0

            h = {}          # (chunk, i) -> h tile
            pend = None     # chunk whose down-groups still need emitting
            emitted = 0
            out_tiles = {}  # (chunk, mblk) -> batched output tile

            def down_group(t, m):
                slot, col0, n, _ = CH[t]
                e = slot
                pd = psd.tile([128, n], F32, tag="pd",
                              padded_shape=[128, CHUNK])
                for ki in range(IT):
                    nc.tensor.matmul(pd[:],
                                     dk[e][:, ki, m * 128:(m + 1) * 128],
                                     h[(t, ki)][:],
                                     start=(ki == 0), stop=(ki == IT - 1))
                blk = m // 2
                if m % 2 == 0:
                    out_tiles[(t, blk)] = op.tile(
                        [128, 2, n], BF16, tag="o", name=f"o{t}_{blk}",
                        padded_shape=[128, 2, CHUNK])
                ot = out_tiles[(t, blk)]
                nc.vector.tensor_copy(ot[:, m % 2, :], pd[:])
                if m % 2 == 1:
                    nc.gpsimd.dma_start(
                        yt[blk * 256:(blk + 1) * 256,
                           col0:col0 + n].rearrange(
                               "(g p) n -> p g n", p=128),
                        ot[:])
                    del out_tiles[(t, blk)]

            def emit_downs(upto):
                nonlocal emitted
                if pend is None:
                    return
                while emitted < upto:
                    down_group(pend, emitted)
                    emitted += 1

            # down-groups of chunk t-1 emitted after each group of chunk t:
            # 2 after each gate group g(1)..g(5), 2 after each up group
            # u(0)..u(2)  -> 16 total
            G_SCHED = [0, 2, 4, 6, 8, 10]
            U_SCHED = [12, 14, 16, 16, 16, 16]

            for t in range(NCH):
                slot, col0, n, first = CH[t]
                e = slot

                if first and e + 1 < EPC:
                    load_weights(e + 1, nc.sync)

                # ---- gate phase ----
                for i in range(IT):
                    pg = psg.tile([128, n], F32, tag="pg",
                                  padded_shape=[128, CHUNK])
                    for k in range(KT):
                        nc.tensor.matmul(pg[:],
                                         gt[e][:, k, i * 128:(i + 1) * 128],
                                         xtl[t][:, k, :],
                                         start=(k == 0), stop=(k == KT - 1))
                    st = sp.tile([128, n], F32, tag="s",
                                 padded_shape=[128, CHUNK])
                    nc.scalar.activation(st[:], pg[:], SILU)
                    h[(t, i)] = st  # placeholder; replaced after mul
                    emit_downs(G_SCHED[i])
                sts = [h[(t, i)] for i in range(IT)]

                # ---- up phase ----
                for i in range(IT):
                    pu = psu.tile([128, n], F32, tag="pu",
                                  padded_shape=[128, CHUNK])
                    for k in range(KT):
                        nc.tensor.matmul(pu[:],
                                         ut[e][:, k, i * 128:(i + 1) * 128],
                                         xtl[t][:, k, :],
                                         start=(k == 0), stop=(k == KT - 1))
                    ht = hp.tile([128, n], BF16, tag="h",
                                 padded_shape=[128, CHUNK])
                    nc.vector.tensor_mul(ht[:], sts[i][:], pu[:])
                    h[(t, i)] = ht
                    emit_downs(U_SCHED[i])

                # x prefetch AFTER this chunk's scalar-queue ops: the DMA may
                # wait on the x(t) buffer free (u-group(t,5)); emitting it
                # earlier would head-of-line block silus -> deadlock against
                # the PSUM-bank WAR dependency.
                if t + 2 < NCH:
                    load_x(t + 2, nc.scalar)

                emit_downs(KT)
                if pend is not None:
                    for ki in range(IT):
                        del h[(pend, ki)]
                pend = t
                emitted = 0

            # tail: down-groups of the final chunk
            for m in range(KT):
                down_group(pend, m)

    nc.compile()
    return nc, ptot


def _get_program(padded):
    key = tuple(padded)
    if key not in _cache:
        _cache[key] = _build(padded)
    return _cache[key]


def _invoke(x, gate_proj, up_proj, down_proj, num_tokens_per_expert,
            trace=False, trace_kwargs=None):
    from concourse.bass_utils import run_bass_kernel_spmd

    x = np.asarray(x)
    counts = np.asarray(num_tokens_per_expert).astype(np.int64)
    assert counts.shape == (NUM_EXPERTS,)
    starts = np.zeros(NUM_EXPERTS + 1, dtype=np.int64)
    np.cumsum(counts, out=starts[1:])

    # per-slot padded counts (max over cores) -> one SPMD program
    cmat = counts.reshape(N_CORES, EPC)
    padded = [int(cmat[:, j].max()) for j in range(EPC)]
    offs = np.zeros(EPC + 1, dtype=np.int64)
    np.cumsum(np.asarray(padded), out=offs[1:])
    ptot_expected = int(offs[-1])

    nc, ptot = _get_program(padded)
    assert ptot == ptot_expected

    gb = np.asarray(gate_proj).astype(BF16_NP)
    ub = np.asarray(up_proj).astype(BF16_NP)
    db = np.asarray(down_proj).astype(BF16_NP)

    in_maps = []
    for c in range(N_CORES):
        xtc = np.zeros((HID, ptot), dtype=BF16_NP)
        for j in range(EPC):
            e = c * EPC + j
            cnt = int(counts[e])
            if cnt:
                xtc[:, int(offs[j]):int(offs[j]) + cnt] = \
                    x[int(starts[e]):int(starts[e]) + cnt].astype(BF16_NP).T
        in_maps.append({
            "xt": xtc,
            "gw": gb[c * EPC:(c + 1) * EPC],
            "uw": ub[c * EPC:(c + 1) * EPC],
            "dw": db[c * EPC:(c + 1) * EPC],
        })

    res = run_bass_kernel_spmd(nc, in_maps, list(range(N_CORES)),
                               trace=trace, **(trace_kwargs or {}))

    out = np.empty((int(starts[-1]), HID), dtype=np.float32)
    for c in range(N_CORES):
        ytc = res.results[c]["yt"]
        for j in range(EPC):
            e = c * EPC + j
            cnt = int(counts[e])
            if cnt:
                out[int(starts[e]):int(starts[e]) + cnt] = \
                    ytc[:, int(offs[j]):int(offs[j]) + cnt].T \
                    .astype(np.float32)
    return out, res


def kernel(x, gate_proj, up_proj, down_proj, num_tokens_per_expert):
    out, _ = _invoke(x, gate_proj, up_proj, down_proj, num_tokens_per_expert)
    return out


# revision 11
# speedup vs baseline: 1.0467x; 1.0170x over previous
"""Grouped-experts MoE (SwiGLU) Bass kernel for Trainium2, 8 NeuronCores.

Expert-parallel: core c owns experts [8c, 8c+8). Tokens are pre-grouped by
expert in the input, so routing is host-side slicing. All device matmuls run
in transposed-token space so every operand streams in its natural layout:

  gateT[i, t] = sum_k G[k, i] * xT[k, t]      (lhsT = G tile, rhs = xT tile)
  hT = silu(gateT) * upT                       (elementwise, [inter, tok])
  outT[m, t] = sum_ki D[ki, m] * hT[ki, t]     (lhsT = D tile, rhs = hT tile)

v3 scheduling (v1 ~1094us, v2 ~1077us):
  - batched mega-DMAs via 3D access patterns: ONE dma per weight matrix per
    expert and ONE per 512-token x chunk (DMA rings process each dma_start
    serially at ~0.6us issue cost; v2's 16-instruction bursts paced the PE)
  - all weights fully double-buffered (G/U/D tiles never wait on frees ->
    no expert-boundary stalls)
  - per chunk: gate-phase (6 groups) then up-phase (6 groups): U(0) only
    needed ~20us after first matmul -> shorter cold start; silu overlaps
    g-phase, mul overlaps u-phase
  - down-projection groups of chunk t-1 interleaved 2-at-a-time between
    groups of chunk t; outputs copied to bf16 and stored in 4-m-group
    batched DMAs
  - psg has 3 PSUM banks so a slow silu (stuck behind an x DMA issue on the
    scalar queue) can't stall the PE; 3+2+3 = 8 banks used
  - first expert processes its remainder chunk first (smaller cold-start
    footprint); last expert ends with two 128-token chunks (short tail)
Host transposes x in / out once per core (not on the device clock).
Compute in bf16 with fp32 PSUM accumulation; bf16 output.
"""

import numpy as np
import ml_dtypes

NUM_EXPERTS = 64
HID = 2048
INTER = 768
N_CORES = 8
EPC = NUM_EXPERTS // N_CORES  # experts per core
KT = HID // 128    # 16 k-tiles over hidden
IT = INTER // 128  # 6 tiles over intermediate
CHUNK = 512        # moving-operand free dim per matmul (HW max)

BF16_NP = ml_dtypes.bfloat16

_cache = {}


def _chunks_of(p, first_expert, last_expert):
    """Chunk sizes for one expert's p tokens."""
    sizes = []
    full, rem = divmod(p, CHUNK)
    if first_expert:
        if rem:
            sizes.append(rem)
        sizes += [CHUNK] * full
    else:
        sizes += [CHUNK] * full
        if rem:
            sizes.append(rem)
    out = []
    n0 = 0
    for s in sizes:
        out.append((n0, s))
        n0 += s
    return out


def _chunk_list(padded):
    """[(slot, col0, n, first_of_expert)] over all experts of this core."""
    ch = []
    off = 0
    ne = len(padded)
    for j, p in enumerate(padded):
        for idx, (n0, n) in enumerate(
                _chunks_of(p, j == 0, j == ne - 1)):
            ch.append((j, off + n0, n, idx == 0))
        off += p
    return ch


def _build(padded):
    import concourse.bacc as bacc
    import concourse.mybir as mybir
    import concourse.tile as tile

    BF16 = mybir.dt.bfloat16
    F32 = mybir.dt.float32
    SILU = mybir.ActivationFunctionType.Silu

    ptot = int(sum(padded))
    CH = _chunk_list(padded)
    NCH = len(CH)

    nc = bacc.Bacc("TRN2", target_bir_lowering=False, debug=False,
                   num_devices=N_CORES)

    xt = nc.dram_tensor("xt", [HID, ptot], BF16, kind="ExternalInput")
    gw = nc.dram_tensor("gw", [EPC, HID, INTER], BF16, kind="ExternalInput")
    uw = nc.dram_tensor("uw", [EPC, HID, INTER], BF16, kind="ExternalInput")
    dw = nc.dram_tensor("dw", [EPC, INTER, HID], BF16, kind="ExternalInput")
    yt = nc.dram_tensor("yt", [HID, ptot], BF16, kind="ExternalOutput")

    with tile.TileContext(nc) as tc:
        with (
            tc.tile_pool(name="xp", bufs=2) as xp,     # 2 x 16K/part
            tc.tile_pool(name="gp", bufs=2) as gp,     # 2 x 24K
            tc.tile_pool(name="upl", bufs=2) as upl,   # 2 x 24K
            tc.tile_pool(name="dp", bufs=2) as dp,     # 2 x 24K
            tc.tile_pool(name="hp", bufs=12) as hp,    # 12K
            tc.tile_pool(name="sp", bufs=6) as sp,     # 12K
            tc.tile_pool(name="op", bufs=2) as op,     # 2 x 4K
            tc.tile_pool(name="psg", bufs=3, space="PSUM") as psg,
            tc.tile_pool(name="psu", bufs=2, space="PSUM") as psu,
            tc.tile_pool(name="psd", bufs=3, space="PSUM") as psd,
        ):
            gt = {}   # expert -> [128, KT, INTER] tile
            ut = {}
            dk = {}   # expert -> [128, IT, HID] tile
            xtl = {}  # chunk idx -> [128, KT, n] tile

            def gsrc(e):
                return gw[e].rearrange("(k p) i -> p k i", p=128)

            def usrc(e):
                return uw[e].rearrange("(k p) i -> p k i", p=128)

            def dsrc(e):
                return dw[e].rearrange("(k p) m -> p k m", p=128)

            def load_weights(e, queue):
                g = gp.tile([128, KT, INTER], BF16, tag="g", name=f"g{e}")
                queue.dma_start(g[:], gsrc(e))
                gt[e] = g
                u = upl.tile([128, KT, INTER], BF16, tag="u", name=f"u{e}")
                queue.dma_start(u[:], usrc(e))
                ut[e] = u
                d = dp.tile([128, IT, HID], BF16, tag="d", name=f"d{e}")
                queue.dma_start(d[:], dsrc(e))
                dk[e] = d

            def load_x(t, queue):
                slot, col0, n, _ = CH[t]
                x = xp.tile([128, KT, n], BF16, tag="x", name=f"x{t}",
                            padded_shape=[128, KT, CHUNK])
                queue.dma_start(
                    x[:], xt[:, col0:col0 + n].rearrange(
                        "(k p) n -> p k n", p=128))
                xtl[t] = x

            # ---- cold-start prologue ----
            # The big batched DMAs (one per matrix) are ring-serial and too
            # slow for the critical first expert. Use per-k-tile DMAs spread
            # round-robin over all three trigger queues so many rings run in
            # parallel, ordered by need: G0+x0 first, then U0, x1, D0.
            QS = [nc.sync, nc.scalar, nc.gpsimd]
            g0 = gp.tile([128, KT, INTER], BF16, tag="g", name="g0")
            gs = gsrc(0)
            u0 = upl.tile([128, KT, INTER], BF16, tag="u", name="u0")
            us = usrc(0)
            slot0, col0_0, n_0, _ = CH[0]
            x0 = xp.tile([128, KT, n_0], BF16, tag="x", name="x0",
                         padded_shape=[128, KT, CHUNK])
            xs0 = xt[:, col0_0:col0_0 + n_0].rearrange("(k p) n -> p k n",
                                                       p=128)
            q = 0
            for k in range(KT):
                QS[q % 3].dma_start(g0[:, k, :], gs[:, k, :]); q += 1
                QS[q % 3].dma_start(x0[:, k, :], xs0[:, k, :]); q += 1
            gt[0] = g0
            xtl[0] = x0
            for k in range(KT):
                QS[q % 3].dma_start(u0[:, k, :], us[:, k, :]); q += 1
            ut[0] = u0
            load_x(1, nc.sync)
            d0 = dp.tile([128, IT, HID], BF16, tag="d", name="d0")
            nc.gpsimd.dma_start(d0[:], dsrc(0))
            dk[0] = d0

            h = {}          # (chunk, i) -> h tile
            pend = None     # chunk whose down-groups still need emitting
            emitted = 0
            out_tiles = {}  # (chunk, mblk) -> batched output tile

            def down_group(t, m):
                slot, col0, n, _ = CH[t]
                e = slot
                pd = psd.tile([128, n], F32, tag="pd",
                              padded_shape=[128, CHUNK])
                for ki in range(IT):
                    nc.tensor.matmul(pd[:],
                                     dk[e][:, ki, m * 128:(m + 1) * 128],
                                     h[(t, ki)][:],
                                     start=(ki == 0), stop=(ki == IT - 1))
                blk = m // 2
                if m % 2 == 0:
                    out_tiles[(t, blk)] = op.tile(
                        [128, 2, n], BF16, tag="o", name=f"o{t}_{blk}",
                        padded_shape=[128, 2, CHUNK])
                ot = out_tiles[(t, blk)]
                nc.vector.tensor_copy(ot[:, m % 2, :], pd[:])
                if m % 2 == 1:
                    nc.gpsimd.dma_start(
                        yt[blk * 256:(blk + 1) * 256,
                           col0:col0 + n].rearrange(
                               "(g p) n -> p g n", p=128),
                        ot[:])
                    del out_tiles[(t, blk)]

            def emit_downs(upto):
                nonlocal emitted
                if pend is None:
                    return
                while emitted < upto:
                    down_group(pend, emitted)
                    emitted += 1

            # down-groups of chunk t-1 emitted after each group of chunk t:
            # 2 after each gate group g(1)..g(5), 2 after each up group
            # u(0)..u(2)  -> 16 total
            G_SCHED = [0, 2, 4, 6, 8, 10]
            U_SCHED = [12, 14, 16, 16, 16, 16]

            for t in range(NCH):
                slot, col0, n, first = CH[t]
                e = slot

                if first and e + 1 < EPC:
                    load_weights(e + 1, nc.sync)

                # ---- gate phase ----
                for i in range(IT):
                    pg = psg.tile([128, n], F32, tag="pg",
                                  padded_shape=[128, CHUNK])
                    for k in range(KT):
                        nc.tensor.matmul(pg[:],
                                         gt[e][:, k, i * 128:(i + 1) * 128],
                                         xtl[t][:, k, :],
                                         start=(k == 0), stop=(k == KT - 1))
                    st = sp.tile([128, n], F32, tag="s",
                                 padded_shape=[128, CHUNK])
                    nc.scalar.activation(st[:], pg[:], SILU)
                    h[(t, i)] = st  # placeholder; replaced after mul
                    emit_downs(G_SCHED[i])
                sts = [h[(t, i)] for i in range(IT)]

                # ---- up phase ----
                for i in range(IT):
                    pu = psu.tile([128, n], F32, tag="pu",
                                  padded_shape=[128, CHUNK])
                    for k in range(KT):
                        nc.tensor.matmul(pu[:],
                                         ut[e][:, k, i * 128:(i + 1) * 128],
                                         xtl[t][:, k, :],
                                         start=(k == 0), stop=(k == KT - 1))
                    ht = hp.tile([128, n], BF16, tag="h",
                                 padded_shape=[128, CHUNK])
                    nc.vector.tensor_mul(ht[:], sts[i][:], pu[:])
                    h[(t, i)] = ht
                    emit_downs(U_SCHED[i])

                # x prefetch AFTER this chunk's scalar-queue ops: the DMA may
                # wait on the x(t) buffer free (u-group(t,5)); emitting it
                # earlier would head-of-line block silus -> deadlock against
                # the PSUM-bank WAR dependency.
                if t + 2 < NCH:
                    load_x(t + 2, nc.scalar)

                emit_downs(KT)
                if pend is not None:
                    for ki in range(IT):
                        del h[(pend, ki)]
                pend = t
                emitted = 0

            # tail: down-groups of the final chunk
            for m in range(KT):
                down_group(pend, m)

    nc.compile()
    return nc, ptot


def _get_program(padded):
    key = tuple(padded)
    if key not in _cache:
        _cache[key] = _build(padded)
    return _cache[key]


def _invoke(x, gate_proj, up_proj, down_proj, num_tokens_per_expert,
            trace=False, trace_kwargs=None):
    from concourse.bass_utils import run_bass_kernel_spmd

    x = np.asarray(x)
    counts = np.asarray(num_tokens_per_expert).astype(np.int64)
    assert counts.shape == (NUM_EXPERTS,)
    starts = np.zeros(NUM_EXPERTS + 1, dtype=np.int64)
    np.cumsum(counts, out=starts[1:])

    # per-slot padded counts (max over cores) -> one SPMD program
    cmat = counts.reshape(N_CORES, EPC)
    padded = [int(cmat[:, j].max()) for j in range(EPC)]
    offs = np.zeros(EPC + 1, dtype=np.int64)
    np.cumsum(np.asarray(padded), out=offs[1:])
    ptot_expected = int(offs[-1])

    nc, ptot = _get_program(padded)
    assert ptot == ptot_expected

    gb = np.asarray(gate_proj).astype(BF16_NP)
    ub = np.asarray(up_proj).astype(BF16_NP)
    db = np.asarray(down_proj).astype(BF16_NP)

    in_maps = []
    for c in range(N_CORES):
        xtc = np.zeros((HID, ptot), dtype=BF16_NP)
        for j in range(EPC):
            e = c * EPC + j
            cnt = int(counts[e])
            if cnt:
                xtc[:, int(offs[j]):int(offs[j]) + cnt] = \
                    x[int(starts[e]):int(starts[e]) + cnt].astype(BF16_NP).T
        in_maps.append({
            "xt": xtc,
            "gw": gb[c * EPC:(c + 1) * EPC],
            "uw": ub[c * EPC:(c + 1) * EPC],
            "dw": db[c * EPC:(c + 1) * EPC],
        })

    res = run_bass_kernel_spmd(nc, in_maps, list(range(N_CORES)),
                               trace=trace, **(trace_kwargs or {}))

    out = np.empty((int(starts[-1]), HID), dtype=np.float32)
    for c in range(N_CORES):
        ytc = res.results[c]["yt"]
        for j in range(EPC):
            e = c * EPC + j
            cnt = int(counts[e])
            if cnt:
                out[int(starts[e]):int(starts[e]) + cnt] = \
                    ytc[:, int(offs[j]):int(offs[j]) + cnt].T \
                    .astype(np.float32)
    return out, res


def kernel(x, gate_proj, up_proj, down_proj, num_tokens_per_expert):
    out, _ = _invoke(x, gate_proj, up_proj, down_proj, num_tokens_per_expert)
    return out
